# revision 20
# baseline (speedup 1.0000x reference)
"""Trainium2 Bass kernel for the MGA dense-transformer block (v3).

Reference computation (per batch n):
    qkv = depthwise3(conv1x1(x, w_qkv), w_dw)         # (3D, L)
    q,k,v per head (dh=64), l2-normalized q,k, scores = q k^T * temp,
    softmax over keys, out = attn @ v, y = conv1x1(out, w_proj)

Sharding over 8 cores: core c -> (batch n = c//2, head group g = c%2 of 4
heads).  Each core computes its 768 qkv channels, runs attention for its 4
heads, and produces a partial projection y_partial (bf16, 512x2048); the
host sums the two partials per batch in f32.

v3 design notes:
  * all matmul operands bf16 (1 cycle / output column on the PE).
  * ACT runs ONLY Exp (the softmax) -> exactly one activation-table load
    for the whole program, no table thrash.  All other elementwise work
    sits on DVE/GPSIMD.
  * depthwise conv: gpsimd drains conv PSUM to bf16 `pre`, DVE does
    center-tap scale + left-tap affine_then_add, gpsimd the right tap.
  * l2 norms: sum-of-squares computed TRANSPOSED on the PE (stationary =
    sq chunk, moving = zero-padded head-select) into one [128,16,8] PSUM
    tile; 1/sqrt via the int-magic + 2 Newton steps entirely on DVE;
    unpacked back to row layout with PE transposes; temperature is folded
    into the broadcast stationary (selbc) host-side.
  * scores transposed (S^T[lk,lq]) into [128,2,512] PSUM tiles; ACT exps
    N=1024 straight from PSUM into bf16 stripes; PV accumulates 16 key
    chunks into po[0:65,:], row 64 (ones column in v^T) is the softmax
    denominator; reciprocal on DVE, broadcast with a K=1 PE matmul.
  * software pipelining: conv+dw, normalize, and v^T build of iteration
    i+1, plus the projection of iteration i-1, are emitted interleaved
    between the 16 attention groups of iteration i, so every engine
    stays fed while ACT grinds through the exps.
  * PSUM budget: sc 2x[128,2,512] + po 2x[128,512] + mi 2x2KB = 16KB.
"""

from contextlib import ExitStack

import numpy as np

import concourse.bacc as bacc
import concourse.mybir as mybir
import concourse.tile as tile
from concourse.bass_utils import run_bass_kernel_spmd

F32 = mybir.dt.float32
F32R = mybir.dt.float32r
BF = mybir.dt.bfloat16
I32 = mybir.dt.int32
F8 = mybir.dt.float8e4
DROW = mybir.MatmulPerfMode.DoubleRow
AF = mybir.ActivationFunctionType
MULT = mybir.AluOpType.mult
ADD = mybir.AluOpType.add
RSHIFT = mybir.AluOpType.logical_shift_right
MAGIC = 0x5F3759DF

N, D, L, H = 4, 512, 2048, 8
DH = D // H          # 64 head dim
HPC = H // 2         # 4 heads per core
C = 3 * 256          # 768 shard qkv channels
P = 128
NLT = L // 512       # 4 query tiles (512 wide)
NLC = L // 128       # 16 key chunks
N_CORES = 8


def build_program(debug_dumps=False, repeat=1):
    nc = bacc.Bacc("TRN2", target_bir_lowering=False, debug=False)

    x_d = nc.dram_tensor("x", (D, L), BF, kind="ExternalInput")
    wqkvT_d = nc.dram_tensor("wqkvT", (D, C), BF, kind="ExternalInput")
    wdw_d = nc.dram_tensor("wdw", (C, 3), F32, kind="ExternalInput")
    wc1_d = nc.dram_tensor("wc1", (6, P, 512), BF, kind="ExternalInput")
    wp2_d = nc.dram_tensor("wp2", (2, P, D), BF, kind="ExternalInput")
    sel8_d = nc.dram_tensor("sel8", (P, 2, 8), BF, kind="ExternalInput")
    selbc_d = nc.dram_tensor("selbc", (8, 2, P), BF, kind="ExternalInput")
    ident_d = nc.dram_tensor("ident", (P, DH), BF, kind="ExternalInput")
    identT_d = nc.dram_tensor("identT", (P, P), F32, kind="ExternalInput")
    ones164_d = nc.dram_tensor("ones164", (1, DH), F32, kind="ExternalInput")
    y_d = nc.dram_tensor("y", (D, L), BF, kind="ExternalOutput")
    dbg = {}
    if debug_dumps:
        dbg["pre0"] = nc.dram_tensor("dbg_pre0", (P, L), BF, kind="ExternalOutput")
        dbg["dw0"] = nc.dram_tensor("dbg_dw0", (P, L), BF, kind="ExternalOutput")
        dbg["dw4"] = nc.dram_tensor("dbg_dw4", (P, L), BF, kind="ExternalOutput")
        dbg["fqk"] = nc.dram_tensor("dbg_fqk", (8, L), BF, kind="ExternalOutput")
        dbg["q8"] = nc.dram_tensor("dbg_q8", (P, 2, L), F8, kind="ExternalOutput")
        dbg["k8"] = nc.dram_tensor("dbg_k8", (P, 2, L), F8, kind="ExternalOutput")
        dbg["vt0"] = nc.dram_tensor("dbg_vt0", (P, NLC, DH + 1), BF, kind="ExternalOutput")
        dbg["st0"] = nc.dram_tensor("dbg_st0", (P, 2, 512), BF, kind="ExternalOutput")
        dbg["outn0"] = nc.dram_tensor("dbg_outn0", (P, L), BF, kind="ExternalOutput")

    with tile.TileContext(nc) as tc, ExitStack() as ctx:
        wp = ctx.enter_context(tc.tile_pool(name="w", bufs=1))
        xp = ctx.enter_context(tc.tile_pool(name="xp", bufs=8))
        prep = ctx.enter_context(tc.tile_pool(name="prep", bufs=2))
        dwp = ctx.enter_context(tc.tile_pool(name="dwp", bufs=2))
        sqp = ctx.enter_context(tc.tile_pool(name="sqp", bufs=8))
        fqp = ctx.enter_context(tc.tile_pool(name="fqp", bufs=2))
        q8p = ctx.enter_context(tc.tile_pool(name="q8p", bufs=2))
        nwp = ctx.enter_context(tc.tile_pool(name="nwp", bufs=3))
        stp = ctx.enter_context(tc.tile_pool(name="stp", bufs=4))
        vtp = ctx.enter_context(tc.tile_pool(name="vtp", bufs=2))
        onp = ctx.enter_context(tc.tile_pool(name="onp", bufs=2))
        ysp = ctx.enter_context(tc.tile_pool(name="ysp", bufs=4))
        rcp = ctx.enter_context(tc.tile_pool(name="rcp", bufs=2))
        scp = ctx.enter_context(tc.tile_pool(name="scp", bufs=2, space="PSUM"))
        pop = ctx.enter_context(tc.tile_pool(name="pop", bufs=2, space="PSUM"))
        mip = ctx.enter_context(tc.tile_pool(name="mip", bufs=2, space="PSUM"))

        # ---- weights / constants -------------------------------------------
        wq_sb = []
        for kc in range(4):
            t = wp.tile([P, C], BF, tag=f"wq{kc}")
            nc.gpsimd.dma_start(t[:], wqkvT_d[kc * 128:(kc + 1) * 128, :])
            wq_sb.append(t)
        wdw_sb = []
        wc1_sb = []
        for cc in range(6):
            t = wp.tile([P, 3], F32, tag=f"wdw{cc}")
            nc.gpsimd.dma_start(t[:], wdw_d[cc * 128:(cc + 1) * 128, :])
            wdw_sb.append(t)
            t2 = wp.tile([P, 512], BF, tag=f"wc1{cc}")
            nc.gpsimd.dma_start(t2[:], wc1_d[cc, :, :])
            wc1_sb.append(t2)
        wp2_sb = []
        for pr in range(2):
            t = wp.tile([P, D], BF, tag=f"wp{pr}")
            nc.gpsimd.dma_start(t[:], wp2_d[pr, :, :])
            wp2_sb.append(t)
        sel8_sb = wp.tile([P, 2, 8], BF, tag="sel8")
        nc.gpsimd.dma_start(sel8_sb[:], sel8_d[:])
        selbc_sb = wp.tile([8, 2, P], BF, tag="selbc")
        nc.gpsimd.dma_start(selbc_sb[:], selbc_d[:])
        ident_sb = wp.tile([P, DH], BF, tag="ident")
        nc.gpsimd.dma_start(ident_sb[:], ident_d[:])
        identT_sb = wp.tile([P, P], F32, tag="identT")
        nc.gpsimd.dma_start(identT_sb[:], identT_d[:])
        ones164_sb = wp.tile([1, DH], F32R, tag="ones164")
        nc.gpsimd.dma_start(ones164_sb[:], ones164_d[:])

        x_tiles = {}     # rep -> [4 tiles]
        dw_tiles = {}    # rep -> [6 tiles]
        vt_tiles = {}    # (rep, hl) -> tile
        qk8_tiles = {}   # rep -> (q8, k8)
        outn_tiles = {}  # rep -> [2 tiles]

        def load_x(rep):
            ts = []
            for kc in range(4):
                t = xp.tile([P, L], BF, tag="x", name=f"x{rep}_{kc}")
                nc.sync.dma_start(t[:], x_d[kc * 128:(kc + 1) * 128, :])
                ts.append(t)
            x_tiles[rep] = ts

        # ---- pipelined phases return lists of closures ---------------------
        def conv_dw_steps(rep):
            dws = [
                dwp.tile([P, L], BF, tag=f"dw{cc}", name=f"dw{rep}_{cc}")
                for cc in range(6)
            ]
            dw_tiles[rep] = dws
            steps = []
            for cc in range(6):
                pre = prep.tile([P, L], BF, tag="pre", name=f"pre{rep}_{cc}")

                ps_box = [None]

                def cstep(cc, lt, kc, pre):
                    if kc == 0:
                        ps_box[0] = mip.tile(
                            [P, 512], F32, tag="mi", name=f"cps{rep}_{cc}_{lt}"
                        )
                    nc.tensor.matmul(
                        ps_box[0][:],
                        wq_sb[kc][:, cc * 128:(cc + 1) * 128],
                        x_tiles[rep][kc][:, lt * 512:(lt + 1) * 512],
                        start=(kc == 0),
                        stop=(kc == 3),
                    )
                    if kc == 3:
                        nc.vector.tensor_copy(
                            pre[:, lt * 512:(lt + 1) * 512], ps_box[0][:]
                        )

                for lt in range(NLT):
                    for kc in range(4):
                        steps.append(
                            (213, lambda cc=cc, lt=lt, kc=kc, pre=pre: cstep(cc, lt, kc, pre))
                        )

                def center(cc, lt, pre, dw):
                    nc.gpsimd.tensor_mul(
                        dw[:, lt * 512:(lt + 1) * 512],
                        pre[:, lt * 512:(lt + 1) * 512],
                        wc1_sb[cc][:],
                    )

                def taps(cc=cc, pre=pre, dw=dws[cc]):
                    w = wdw_sb[cc]
                    nc.vector.affine_then_add(
                        dw[:, 1:L], pre[:, 0:L - 1], dw[:, 1:L],
                        scale=w[:, 0:1], bias=0.0,
                    )
                    nc.vector.scalar_tensor_tensor(
                        dw[:, 0:L - 1], pre[:, 1:L], w[:, 2:3], dw[:, 0:L - 1],
                        op0=MULT, op1=ADD,
                    )

                for lt in range(NLT):
                    steps.append(
                        (0, lambda cc=cc, lt=lt, pre=pre, dw=dws[cc]: center(cc, lt, pre, dw))
                    )
                steps.append((0, taps))

                def dump_dw(cc=cc, pre=pre, dw=dws[cc]):
                    if cc == 0:
                        nc.sync.dma_start(dbg["pre0"][:], pre[:])
                        nc.sync.dma_start(dbg["dw0"][:], dw[:])
                    if cc == 4:
                        nc.sync.dma_start(dbg["dw4"][:], dw[:])

                if debug_dumps and rep == 0 and cc in (0, 4):
                    steps.append((0, dump_dw))
            return steps

        def normalize_steps(rep):
            # Transposed sum-of-squares: packT[c, i, h] = sum_ch sq_v[ch, lq]
            # for lq = i*128 + c, head-row h; then rsqrt on DVE via the
            # int-magic + 2 Newton iterations; unpack via PE transposes.
            dws = dw_tiles[rep]
            steps = []
            yfin = [None]
            fqk = fqp.tile([8, L], BF, tag="fqk", name=f"fqk{rep}")
            qk8_tiles[rep] = (
                q8p.tile([P, 2, L], F8, tag="q8", name=f"q8_{rep}"),
                q8p.tile([P, 2, L], F8, tag="k8", name=f"k8_{rep}"),
            )
            s_sb = nwp.tile([P, NLC, 8], F32, tag="ssb", name=f"ssb{rep}")
            packT = [None]

            sq_tiles = {}

            def sq_step(lt, v):
                sq = sqp.tile([P, 512], BF, tag="sq", name=f"sq{rep}_{lt}_{v}")
                s = dws[v][:, lt * 512:(lt + 1) * 512]
                nc.gpsimd.tensor_mul(sq[:], s, s)
                sq_tiles[(lt, v)] = sq

            def norm_mm(lt):
                # accumulation groups must be CONSECUTIVE per psum slice:
                # v-interleaved groups lose the start contribution on HW
                pk = mip.tile([P, 4, 8], F32, tag="mi", name=f"packT{rep}_{lt}")
                for i in range(4):
                    for v in range(4):
                        nc.tensor.matmul(
                            pk[:, i, :],
                            sq_tiles[(lt, v)][:, i * 128:(i + 1) * 128],
                            sel8_sb[:, v // 2, :],
                            start=(v == 0),
                            stop=(v == 3),
                        )
                nc.vector.tensor_copy(s_sb[:, lt * 4:lt * 4 + 4, :], pk[:])
                for v in range(4):
                    sq_tiles.pop((lt, v))

            for lt in range(NLT):
                for v in range(4):
                    steps.append((10, lambda lt=lt, v=v: sq_step(lt, v)))
                steps.append((130, lambda lt=lt: norm_mm(lt)))

            def rsqrt_seed():
                t1 = nwp.tile([P, NLC, 8], I32, tag="nw", name=f"nt1{rep}")
                nc.vector.tensor_scalar(t1[:], s_sb.bitcast(I32)[:], 1, None, op0=RSHIFT)
                y0 = nwp.tile([P, NLC, 8], I32, tag="nw", name=f"ny0{rep}")
                nc.vector.tensor_scalar(y0[:], t1[:], -1, MAGIC, op0=MULT, op1=ADD)
                return y0.bitcast(F32)

            def newton(y):
                a = nwp.tile([P, NLC, 8], F32, tag="nw", name=f"na{rep}_{id(y) % 97}")
                nc.vector.tensor_mul(a[:], y[:], y[:])
                nc.vector.tensor_mul(a[:], a[:], s_sb[:])
                nc.vector.tensor_scalar(a[:], a[:], -0.5, 1.5, op0=MULT, op1=ADD)
                out = nwp.tile([P, NLC, 8], F32, tag="nw", name=f"ny{rep}_{id(y) % 97}")
                nc.vector.tensor_mul(out[:], a[:], y[:])
                return out

            def rsqrt_all():
                y0 = rsqrt_seed()
                y1 = newton(y0)
                yfin[0] = newton(y1)

            steps.append((0, rsqrt_all))

            def unpack(lt):
                up = mip.tile([8, 4, P], F32, tag="mi", name=f"up{rep}_{lt}")
                for i in range(4):
                    nc.tensor.matmul(
                        up[:, i, :],
                        yfin[0][:, lt * 4 + i, :],
                        identT_sb[:],
                        is_transpose=True, start=True, stop=True,
                    )
                nc.vector.tensor_copy(fqk[:, lt * 512:(lt + 1) * 512], up[:])

            for lt in range(NLT):
                steps.append((430, lambda lt=lt: unpack(lt)))

            q8, k8 = qk8_tiles[rep]

            def bc_apply(qk, lt):
                bc = mip.tile([P, 512], F32, tag="mi", name=f"bc{rep}_{qk}_{lt}")
                nc.tensor.matmul(
                    bc[:], selbc_sb[:, qk, :], fqk[:, lt * 512:(lt + 1) * 512],
                    start=True, stop=True,
                )
                dst8 = q8 if qk == 0 else k8
                for j in range(2):
                    nc.vector.tensor_mul(
                        dst8[:, j, lt * 512:(lt + 1) * 512],
                        dws[2 * qk + j][:, lt * 512:(lt + 1) * 512],
                        bc[:],
                    )

            for qk in range(2):
                for lt in range(NLT):
                    steps.append((215, lambda qk=qk, lt=lt: bc_apply(qk, lt)))

            def dump_norm():
                nc.sync.dma_start(dbg["fqk"][:], fqk[:])
                nc.sync.dma_start(dbg["q8"][:], q8[:])
                nc.sync.dma_start(dbg["k8"][:], k8[:])

            if debug_dumps and rep == 0:
                steps.append((0, dump_norm))
            return steps

        def build_vt_steps(rep):
            dws = dw_tiles[rep]
            steps = []
            for hl in range(HPC):
                vt = vtp.tile(
                    [P, NLC, DH + 1], BF, tag=f"vt{hl}", name=f"vt{rep}_{hl}"
                )
                vt_tiles[(rep, hl)] = vt
                steps.append(
                    (0, lambda vt=vt: nc.vector.memset(vt[:, :, DH:DH + 1], 1.0))
                )

                def tgroup(hl, lg, vt):
                    vsl = dws[4 + hl // 2]
                    b = DH * (hl % 2)
                    tp = mip.tile([P, 4, DH], BF, tag="mi", name=f"tp{rep}_{hl}_{lg}")
                    for j in range(4):
                        lc = 4 * lg + j
                        nc.tensor.matmul(
                            tp[:, j, :],
                            vsl[b:b + DH, lc * 128:(lc + 1) * 128],
                            ident_sb[b:b + DH, :],
                            is_transpose=True, start=(j == 0), stop=(j == 3),
                        )
                    nc.vector.tensor_copy(vt[:, 4 * lg:4 * lg + 4, 0:DH], tp[:])

                for lg in range(NLC // 4):
                    steps.append((120, lambda hl=hl, lg=lg, vt=vt: tgroup(hl, lg, vt)))
                if debug_dumps and rep == 0 and hl == 0:
                    steps.append(
                        (0, lambda vt=vt: nc.sync.dma_start(dbg["vt0"][:], vt[:]))
                    )
            return steps

        def proj_steps(rep):
            outn = outn_tiles[rep]
            steps = []
            for oc in range(4):
                ysb = ysp.tile([P, L], BF, tag="ysb", name=f"ysb{rep}_{oc}")

                def pstep(oc, lt, ysb):
                    pp = pop.tile([P, 512], F32, tag="po", name=f"yps{rep}_{oc}_{lt}")
                    for pr in range(2):
                        nc.tensor.matmul(
                            pp[:],
                            wp2_sb[pr][:, oc * 128:(oc + 1) * 128],
                            outn[pr][:, lt * 512:(lt + 1) * 512],
                            start=(pr == 0), stop=(pr == 1),
                        )
                    nc.vector.tensor_copy(ysb[:, lt * 512:(lt + 1) * 512], pp[:])
                    if lt == NLT - 1:
                        nc.sync.dma_start(y_d[oc * 128:(oc + 1) * 128, :], ysb[:])

                for lt in range(NLT):
                    steps.append((430, lambda oc=oc, lt=lt, ysb=ysb: pstep(oc, lt, ysb)))
            return steps

        def attention(rep, extra_steps):
            q8, k8 = qk8_tiles[rep]
            outn_tiles[rep] = [
                onp.tile([P, L], BF, tag=f"outn{pr}", name=f"outn{rep}_{pr}")
                for pr in range(2)
            ]
            si = 0
            n_groups = HPC * NLT
            total_cost = sum(c for c, _ in extra_steps) or 1
            cum = 0

            def drain(po, hl, lt):
                # softmax denominator: row DH of po holds sum(exp); divide
                # the other 64 rows by it via DVE recip + K=1 broadcast mm.
                # Deferred into the NEXT group so the bcd matmul never
                # blocks the PE queue right at a group boundary.
                rec = rcp.tile(
                    [1, 512], F32R, tag="rec", name=f"rec{rep}_{hl}_{lt}"
                )
                with nc.allow_low_precision(reason="1/denom feeds f32r matmul"):
                    nc.vector.reciprocal(rec[:], po[DH:DH + 1, :])
                bcd = mip.tile([DH, 512], F32, tag="mi", name=f"bcd{rep}_{hl}_{lt}")
                nc.tensor.matmul(
                    bcd[:], ones164_sb[:], rec[:], start=True, stop=True
                )
                bcs = rcp.tile([DH, 512], F32, tag="bcs", name=f"bcs{rep}_{hl}_{lt}")
                nc.vector.tensor_copy(bcs[:], bcd[:])
                dst = outn_tiles[rep][hl // 2][
                    DH * (hl % 2):DH * (hl % 2) + DH, lt * 512:(lt + 1) * 512
                ]
                nc.vector.tensor_mul(dst, po[0:DH, :], bcs[:])

            pending_drain = None
            for hl in range(HPC):
                qrow = q8[32 * hl:32 * hl + 32, :, :]
                krow = k8[32 * hl:32 * hl + 32, :, :]
                vt = vt_tiles[(rep, hl)]
                for lt in range(NLT):
                    po = pop.tile([P, 512], F32, tag="po", name=f"po{rep}_{hl}_{lt}")
                    for g in range(NLC // 2):
                        sc = scp.tile(
                            [P, 2, 512], F32, tag="sc", name=f"sc{rep}_{hl}_{lt}_{g}"
                        )
                        for j in range(2):
                            lc = 2 * g + j
                            nc.tensor.matmul(
                                sc[:, j, :],
                                krow[:, :, lc * 128:(lc + 1) * 128],
                                qrow[:, :, lt * 512:(lt + 1) * 512],
                                start=True, stop=True, perf_mode=DROW,
                                tile_position=(32 * hl, 0),
                            )
                        st = stp.tile(
                            [P, 2, 512], BF, tag="st", name=f"st{rep}_{hl}_{lt}_{g}"
                        )
                        nc.scalar.activation(st[:], sc[:], AF.Exp)
                        if debug_dumps and rep == 0 and hl == 0 and lt == 0 and g == 0:
                            nc.sync.dma_start(dbg["st0"][:], st[:])
                        for j in range(2):
                            lc = 2 * g + j
                            nc.tensor.matmul(
                                po[0:DH + 1, :],
                                vt[:, lc, 0:DH + 1],
                                st[:, j, :],
                                start=(lc == 0), stop=(lc == NLC - 1),
                            )
                        if g == 1 and pending_drain is not None:
                            drain(*pending_drain)
                            pending_drain = None
                        # interleave pipelined work (next iter's conv/norm/
                        # vt, previous iter's projection) INSIDE the group
                        ui = hl * NLT * 8 + lt * 8 + g
                        want = (ui + 1) * total_cost // (n_groups * 8)
                        while si < len(extra_steps) and cum < want:
                            cum += extra_steps[si][0]
                            extra_steps[si][1]()
                            si += 1
                    pending_drain = (po, hl, lt)
            while si < len(extra_steps):
                extra_steps[si][1]()
                si += 1
            if pending_drain is not None:
                drain(*pending_drain)
            if debug_dumps and rep == 0:
                nc.sync.dma_start(dbg["outn0"][:], outn_tiles[rep][0][:])

        # ---- schedule ------------------------------------------------------
        load_x(0)
        for _, s in conv_dw_steps(0) + normalize_steps(0) + build_vt_steps(0):
            s()
        pend = []
        for rep in range(repeat):
            nxt = []
            if rep + 1 < repeat:
                load_x(rep + 1)
                nxt = (
                    conv_dw_steps(rep + 1)
                    + normalize_steps(rep + 1)
                    + build_vt_steps(rep + 1)
                )
            attention(rep, pend + nxt)
            pend = proj_steps(rep)
            if rep > 0:
                dw_tiles.pop(rep - 1, None)
                x_tiles.pop(rep - 1, None)
                outn_tiles.pop(rep - 1, None)
        for _, s in pend:
            s()

    nc.compile()
    return nc


def make_in_maps(x, w_qkv, w_dw, w_proj, temperature):
    bf = mybir.dt.np(BF)
    x = np.asarray(x, dtype=np.float32)
    w_qkv = np.asarray(w_qkv, dtype=np.float32)
    w_dw = np.asarray(w_dw, dtype=np.float32)
    w_proj = np.asarray(w_proj, dtype=np.float32)
    temperature = np.asarray(temperature, dtype=np.float32)

    # q,k channels live in "plane" layout for fp8 DoubleRow: head hl's
    # channel dh sits at partition 32*hl + dh%32 of plane j = dh//32.
    sel8 = np.zeros((P, 2, 8), np.float32)
    for r in range(P):
        sel8[r, 0, r // 32] = 1.0        # q planes -> norm rows 0..3
        sel8[r, 1, 4 + r // 32] = 1.0    # k planes -> norm rows 4..7
    ident = np.vstack([np.eye(DH, dtype=np.float32)] * 2)
    identT = np.eye(P, dtype=np.float32)

    # per-core channel row order: q plane0, q plane1, k plane0, k plane1,
    # v (natural order)
    def plane_rows(base):
        out = []
        for j in range(2):
            for hl in range(HPC):
                out.extend(base + 64 * hl + 32 * j + np.arange(32))
        return np.array(out)

    in_maps = []
    for c in range(N_CORES):
        n, g = c // 2, c % 2
        rows = np.concatenate(
            [
                plane_rows(256 * g),
                plane_rows(512 + 256 * g),
                1024 + 256 * g + np.arange(256),
            ]
        )
        t_g = temperature[0, HPC * g:HPC * g + HPC, 0, 0]
        # broadcast stationary; q rows carry the temperature factor
        selbc = np.zeros((8, 2, P), np.float32)
        for r in range(P):
            selbc[r // 32, 0, r] = t_g[r // 32]
            selbc[4 + r // 32, 1, r] = 1.0
        wp2 = np.stack(
            [
                w_proj[:, 256 * g + 128 * pr:256 * g + 128 * (pr + 1), 0].T
                for pr in range(2)
            ]
        )
        wc1 = np.repeat(w_dw[rows[:768], 0, 1][:, None], 512, axis=1).reshape(
            6, P, 512
        )
        in_maps.append(
            {
                "x": x[n].astype(bf),
                "wqkvT": np.ascontiguousarray(w_qkv[rows, :, 0].T).astype(bf),
                "wdw": np.ascontiguousarray(w_dw[rows, 0, :]),
                "wc1": wc1.astype(bf),
                "wp2": np.ascontiguousarray(wp2).astype(bf),
                "sel8": sel8.astype(bf),
                "selbc": selbc.astype(bf),
                "ident": ident.astype(bf),
                "identT": identT,
                "ones164": np.ones((1, DH), np.float32),
            }
        )
    return in_maps


_PROGRAM = None


def _get_program():
    global _PROGRAM
    if _PROGRAM is None:
        _PROGRAM = build_program()
    return _PROGRAM


def kernel(x, w_qkv, w_dw, w_proj, temperature):
    prog = _get_program()
    in_maps = make_in_maps(x, w_qkv, w_dw, w_proj, temperature)
    res = run_bass_kernel_spmd(prog, in_maps, list(range(N_CORES)))
    y = np.empty((N, D, L), np.float32)
    for n in range(N):
        y[n] = res.results[2 * n]["y"].astype(np.float32) + res.results[
            2 * n + 1
        ]["y"].astype(np.float32)
    return y


if __name__ == "__main__":
    prog = build_program()
    print("program built ok")
